# revision 37
# baseline (speedup 1.0000x reference)
"""Cubic B-spline evaluation (uniform knots) on 8 Trainium2 NeuronCores.

v7: qf-based two-phase pipeline.  Spline pair index q = floor(x/2) in [1,31];
on pair q the spline is a cubic in v = x - 2q in [0,2):
  out = HC(v) + rr * HD(v),  rr = 1{v >= 1}
with 32-entry tables C, D (host-derived).  Lookups are step sums over
thresholds 1{qf >= i - 0.5} built by a K=5 bf16 matmul (MM1) over 4 point
slots, an indicator pass (ACT sign / DVE is_ge), and a contraction (MM2)
with bf16 hi+lo difference weights.  Table octets move PSUM->SBUF via the
hi+lo merge op itself (ACT copy after PSUM accumulation, or DVE add of two
column blocks), then transpose to pointwise layout through DRAM scratch.

Two f-phases: chunks 0-7 produce g for point-columns [0,512) (all 128
partitions), chunks 8-15 for [512,1024).  Phase-0 g loads issue at chunk 8
and the phase-0 Horner interleaves with chunks 9-15, so only the phase-1
Horner remains as tail.

Layout (per core, N = 131072): pointwise x_pw[p, f] = x[1024 p + f],
p = 32 s + q.  Group (t, tau): phase H = t//8, tt = t%8, q = 4 tt + tau,
covers points (p = 32 s + q, f = 512 H + c), c in [0,512).
"""

import sys

sys.path.insert(0, "/opt/trn_rl_repo")

import numpy as np

N_TOTAL = 1_048_576
N_CORES = 8
N = N_TOTAL // N_CORES  # 131072 points per core
P = 128
COLS = N // P  # 1024
TW = 512
NCHUNK = 16


def _eng_of_q(q: int) -> str:
    return "act" if (3 * q) % 5 < 3 else "dve"


def _gamma_vec(gamma_k: np.ndarray) -> np.ndarray:
    g = np.zeros((P, 8), np.float32)
    for p in range(P):
        q = 4 * (p // 16) + (p % 4)  # producing group of partition p
        if _eng_of_q(q) == "act":
            g[p] = gamma_k
    return g


def _tables(coefs: np.ndarray):
    import ml_dtypes

    c = np.zeros(67, np.float64)
    c[3:] = np.asarray(coefs, np.float64)
    jj = np.arange(64)
    a0 = (c[jj] + 4 * c[jj + 1] + c[jj + 2]) / 6
    a1 = (c[jj + 2] - c[jj]) / 2
    a2 = (c[jj] - 2 * c[jj + 1] + c[jj + 2]) / 2
    a3 = (c[jj + 3] - c[jj] + 3 * c[jj + 1] - 3 * c[jj + 2]) / 6
    A = np.stack([a0, a1, a2, a3], 1)  # [64, 4] coeffs in u = x - j

    # rebase odd segments to v = u + 1 (v = x - 2q)
    B = A.copy()
    r1 = jj % 2 == 1
    B[r1, 0] = A[r1, 0] - A[r1, 1] + A[r1, 2] - A[r1, 3]
    B[r1, 1] = A[r1, 1] - 2 * A[r1, 2] + 3 * A[r1, 3]
    B[r1, 2] = A[r1, 2] - 3 * A[r1, 3]
    B[r1, 3] = A[r1, 3]
    C = B[0::2]  # [32, 4]
    D = B[1::2] - B[0::2]  # [32, 4]

    # halved step-difference weights (unified sign/{0,2} convention)
    WC = C.copy()
    WC[1:] -= C[:-1]
    WD = D.copy()
    WD[1:] -= D[:-1]
    Wp = np.concatenate([WC, WD], 1) * 0.5  # [32, 8]: col 4 cd + k
    gamma_k = Wp.sum(0).astype(np.float32)  # [8]

    # MM1 lhsT [4, 128]: col 32 s + i -> psum = qf_s; thr applied in the
    # indicator op as a per-partition scalar
    w1 = np.zeros((4, 128), np.float64)
    thr = np.empty(32)
    thr[0] = -1.0
    thr[1:] = np.arange(1, 32) - 0.5
    for s in range(4):
        w1[s, 32 * s : 32 * s + 32] = 1.0
    # MM2 lhsT [128, 32]: row 32 r + i, col m2 = 16 cd + 4 k + r
    w2 = np.zeros((128, 32), np.float64)
    for r in range(4):
        for cd in range(2):
            for k in range(4):
                w2[32 * r : 32 * r + 32, 16 * cd + 4 * k + r] = Wp[:, 4 * cd + k]
    bf = ml_dtypes.bfloat16
    w2hi = w2.astype(bf)
    w2lo = (w2 - w2hi.astype(np.float64)).astype(bf)
    # pack all constants into one [128, 208] bf16 tensor:
    #   cols   0:128  rows 0:5   w1
    #   cols 128:160  w2hi, cols 160:192  w2lo
    #   cols 192:208  gamma (f32 bit-packed as bf16 pairs)
    pack = np.zeros((128, 212), bf)
    pack[0:4, 0:128] = w1.astype(bf)
    pack[:, 128:160] = w2hi
    pack[:, 160:192] = w2lo
    gvec = _gamma_vec(gamma_k)  # [128, 8] f32
    pack[:, 192:208] = gvec.astype(np.float32).view(np.uint16).view(bf)
    thrv = np.tile(thr, 4).astype(np.float32)  # [128] per-partition
    negthr = (-thrv).astype(np.float32)
    pack[:, 208:210] = thrv.reshape(128, 1).view(np.uint16).view(bf)
    pack[:, 210:212] = negthr.reshape(128, 1).view(np.uint16).view(bf)
    return pack


_PROG_CACHE: dict = {}


def _build_program():
    import concourse.bacc as bacc
    import concourse.mybir as mybir
    from concourse.tile import TileContext

    f32 = mybir.dt.float32
    bf16 = mybir.dt.bfloat16
    Alu = mybir.AluOpType

    nc = bacc.Bacc("TRN2", debug=False)

    x_dram = nc.dram_tensor("x", [N], f32, kind="ExternalInput")
    cpack_dram = nc.dram_tensor("cpack", [P, 212], bf16, kind="ExternalInput")
    out_dram = nc.dram_tensor("out", [N], f32, kind="ExternalOutput")
    qf_dram = nc.dram_tensor("qf_scratch", [2, P * TW], bf16, kind="Internal")
    # g3[H, cd, k, tt, r, tau, fpc]
    g3_dram = nc.dram_tensor(
        "g_scratch", [2, 2, 4, 8, 4, 4, TW], f32, kind="Internal"
    )

    x_view = x_dram.ap().rearrange("(p f) -> p f", p=P)
    out_view = out_dram.ap().rearrange("(p f) -> p f", p=P)
    # qf loads per chunk pair: [H, tp, r, ttsub, (tau fpc)=2048]
    qf_ld = qf_dram.ap().rearrange(
        "H (tp ttsub r tf) -> H tp r ttsub tf", tp=4, ttsub=2, r=4
    )
    # g3 store view: [H, cd, tt, k, r, (tau fpc)]
    g3_st = g3_dram.ap().rearrange(
        "H cd k tt r tau fpc -> H cd tt k r (tau fpc)"
    )

    with TileContext(nc) as tc:
        with (
            tc.tile_pool(name="const", bufs=1) as cpool,
            tc.tile_pool(name="pw", bufs=1) as pw,
            tc.tile_pool(name="tmp", bufs=1) as tmp,
            tc.tile_pool(name="hrn", bufs=1) as hp,
            tc.tile_pool(name="sind", bufs=1) as spool,
            tc.tile_pool(name="gcp", bufs=1) as gcpool,
            tc.tile_pool(name="psum1", bufs=1, space="PSUM") as pp1,
            tc.tile_pool(name="psum2", bufs=1, space="PSUM") as pp2,
        ):
            # ---- constants: one packed DMA; ones rows via pool memset ----
            cpk = cpool.tile([P, 212], bf16, tag="cpk")
            nc.sync.dma_start(out=cpk[:], in_=cpack_dram.ap())
            w1_sb = cpk[0:4, 0:128]
            w2hi_sb = cpk[:, 128:160]
            w2lo_sb = cpk[:, 160:192]
            gam_sb = cpk[:, 192:208].bitcast(f32)
            thr_sb = cpk[:, 208:210].bitcast(f32)
            negthr_sb = cpk[:, 210:212].bitcast(f32)
            j_bufs = []
            for bi in range(3):
                jb = cpool.tile(
                    [4, 4096], bf16, tag=f"jbuf{bi}", name=f"jbuf{bi}"
                )
                j_bufs.append(jb)

            ps1_bufs = [
                pp1.tile([P, TW], f32, tag=f"s1_{i}", name=f"ps1f{i}")
                for i in range(4)
            ]
            ps2_bufs = [
                pp2.tile([32, TW], f32, tag=f"s2_{i}", name=f"ps2f{i}")
                for i in range(4)
            ]
            s_bufs = [
                spool.tile([P, TW], bf16, tag=f"sb_{i}", name=f"sbf{i}")
                for i in range(8)
            ]
            gcp_full = [
                gcpool.tile([32, 4 * TW], f32, tag=f"gc_{i}", name=f"gcpf{i}")
                for i in range(3)
            ]

            # dummies: absorb constant-load DMA sems into the PE vector clock
            pdum = ps1_bufs[0]
            nc.tensor.matmul(
                out=pdum[:, 0:8], lhsT=w1_sb[:], rhs=w1_sb[:, 0:8],
                start=True, stop=True,
            )
            nc.tensor.matmul(
                out=pdum[0:32, 0:8], lhsT=w2hi_sb[:], rhs=w2hi_sb[:, 0:8],
                start=True, stop=True,
            )
            nc.tensor.matmul(
                out=pdum[0:32, 0:8], lhsT=w2lo_sb[:], rhs=w2lo_sb[:, 0:8],
                start=True, stop=True,
            )

            # ---- pointwise prep, split by phase half so chunk 0 can
            # start as soon as the H=0 qf scratch is written ----
            x_pw = pw.tile([P, COLS], f32, tag="x")
            t1_pw = tmp.tile([P, COLS], f32, tag="ta", name="prep_t1")
            qf_pw = pw.tile([P, COLS], bf16, tag="qf")
            v_pw = pw.tile([P, COLS], f32, tag="v")
            rr_pw = pw.tile([P, COLS], f32, tag="rr")
            v2_pw = pw.tile([P, COLS], f32, tag="v2")
            for hh in range(2):
                sl = slice(TW * hh, TW * hh + TW)
                nc.sync.dma_start(out=x_pw[:, sl], in_=x_view[:, sl])
                nc.vector.tensor_scalar(
                    t1_pw[:, sl], x_pw[:, sl], 0.5, 8388607.5,
                    Alu.mult, Alu.add
                )
                nc.vector.tensor_scalar(
                    qf_pw[:, sl], t1_pw[:, sl], -8388608.0, None, Alu.add
                )
                nc.sync.dma_start(
                    out=qf_dram.ap()[hh].rearrange("(p c) -> p c", p=P),
                    in_=qf_pw[:, sl],
                )
                nc.vector.scalar_tensor_tensor(
                    v_pw[:, sl], qf_pw[:, sl], -2.0, x_pw[:, sl],
                    Alu.mult, Alu.add
                )
                nc.vector.tensor_scalar(
                    rr_pw[:, sl], v_pw[:, sl], 1.0, None, Alu.is_ge
                )
                nc.scalar.square(v2_pw[:, sl], v_pw[:, sl])

            g_all = pw.tile([P, 2, 4, COLS], f32, tag="gall")
            res = pw.tile([P, COLS], f32, tag="res")

            # ---- phase Horner op lists (emitted interleaved) ----
            def horner_ops(H):
                c0 = TW * H
                sl = slice(c0, c0 + TW)
                v_ = v_pw[:, sl]
                v2_ = v2_pw[:, sl]
                rr_ = rr_pw[:, sl]
                ops = []
                hres = []
                for cd in range(2):
                    gk = [g_all[:, cd, k, sl] for k in range(4)]
                    g2c = hp.tile([P, TW], f32, tag=f"h{cd}a", name=f"g2c{cd}_{H}")
                    g3c = hp.tile([P, TW], f32, tag=f"h{cd}b", name=f"g3c{cd}_{H}")
                    v1t = hp.tile([P, TW], f32, tag=f"h{cd}c", name=f"v1t{cd}_{H}")
                    v2t = hp.tile([P, TW], f32, tag=f"h{cd}d", name=f"v2t{cd}_{H}")
                    pacc = hp.tile([P, TW], f32, tag=f"h{cd}a", name=f"pacc{cd}_{H}")
                    qacc = hp.tile([P, TW], f32, tag=f"h{cd}b", name=f"qacc{cd}_{H}")
                    v3t = hp.tile([P, TW], f32, tag=f"h{cd}c", name=f"v3t{cd}_{H}")
                    hr = hp.tile([P, TW], f32, tag=f"h{cd}d", name=f"hr{cd}_{H}")
                    e = nc.vector if cd == 0 else nc.gpsimd
                    ops.append(lambda gk=gk, g2c=g2c, cd=cd: nc.scalar.add(
                        g2c[:], gk[2], gam_sb[:, 4 * cd + 2 : 4 * cd + 3]))
                    ops.append(lambda gk=gk, g3c=g3c, cd=cd: nc.scalar.add(
                        g3c[:], gk[3], gam_sb[:, 4 * cd + 3 : 4 * cd + 4]))
                    ops.append(lambda e=e, v1t=v1t, g2c=g2c, v2_=v2_:
                               e.tensor_tensor(out=v1t[:], in0=g2c[:],
                                               in1=v2_, op=Alu.mult))
                    ops.append(lambda e=e, v2t=v2t, g3c=g3c, v2_=v2_:
                               e.tensor_tensor(out=v2t[:], in0=g3c[:],
                                               in1=v2_, op=Alu.mult))
                    ops.append(lambda pacc=pacc, v1t=v1t, gk=gk, cd=cd:
                               nc.vector.scalar_tensor_tensor(
                                   pacc[:], v1t[:],
                                   gam_sb[:, 4 * cd : 4 * cd + 1], gk[0],
                                   Alu.add, Alu.add))
                    ops.append(lambda qacc=qacc, v2t=v2t, gk=gk, cd=cd:
                               nc.vector.scalar_tensor_tensor(
                                   qacc[:], v2t[:],
                                   gam_sb[:, 4 * cd + 1 : 4 * cd + 2], gk[1],
                                   Alu.add, Alu.add))
                    ops.append(lambda e=e, v3t=v3t, qacc=qacc, v_=v_:
                               e.tensor_tensor(out=v3t[:], in0=qacc[:],
                                               in1=v_, op=Alu.mult))
                    ops.append(lambda e=e, hr=hr, pacc=pacc, v3t=v3t:
                               e.tensor_tensor(out=hr[:], in0=pacc[:],
                                               in1=v3t[:], op=Alu.add))
                    hres.append(hr)
                rd = hp.tile([P, TW], f32, tag="h0a", name=f"rd_{H}")
                ops.append(lambda rd=rd, h1=hres[1], rr_=rr_:
                           nc.vector.tensor_tensor(out=rd[:], in0=h1[:],
                                                   in1=rr_, op=Alu.mult))
                ops.append(lambda rd=rd, h0=hres[0], sl=sl:
                           nc.vector.tensor_tensor(out=res[:, sl], in0=h0[:],
                                                   in1=rd[:], op=Alu.add))
                ops.append(lambda sl=sl: nc.sync.dma_start(
                    out=out_view[:, sl], in_=res[:, sl]))
                return ops

            pending = []

            # ---- chunk loop ----
            for t in range(NCHUNK):
                H, tt = t // 8, t % 8
                if t in (4, 12):
                    # early loads: first tt-half of the current phase
                    Hc = t // 8
                    for cd in range(2):
                        for k in range(4):
                            nc.sync.dma_start(
                                out=g_all[0:64, cd, k,
                                          TW * Hc : TW * Hc + TW],
                                in_=g3_dram.ap()[Hc, cd, k, 0:4],
                            )
                if t == 8:
                    # phase-0 remaining loads (tt >= 4)
                    for cd in range(2):
                        for k in range(4):
                            nc.sync.dma_start(
                                out=g_all[64:128, cd, k, 0:TW],
                                in_=g3_dram.ap()[0, cd, k, 4:8],
                            )
                    pending = horner_ops(0)
                j_pk = j_bufs[(t // 2) % 3]
                if t == 0:
                    # pair 0 load (pair i+1 is prefetched a pair ahead below)
                    nc.sync.dma_start(
                        out=j_pk[0:4, :], in_=qf_ld[0, 0]
                    )
                if t % 2 == 0 and t < NCHUNK - 2:
                    pn = t // 2 + 1
                    nc.sync.dma_start(
                        out=j_bufs[pn % 3][0:4, :],
                        in_=qf_ld[pn // 4, pn % 4],
                    )
                gcp = gcp_full[t % 3]
                for tau in range(4):
                    q = 4 * tt + tau
                    gi = 4 * t + tau
                    eng = _eng_of_q(q)
                    ps1 = ps1_bufs[gi % 4]
                    nc.tensor.matmul(
                        out=ps1[:],
                        lhsT=w1_sb[:],
                        rhs=j_pk[
                            :, (t % 2) * 2048 + tau * TW :
                            (t % 2) * 2048 + (tau + 1) * TW
                        ],
                        start=True,
                        stop=True,
                    )
                    s_sb = s_bufs[gi % 8]
                    if eng == "act":
                        nc.scalar.sign(s_sb[:], ps1[:], bias=negthr_sb)  # {-1,+1}
                    else:
                        nc.vector.tensor_scalar(
                            s_sb[:], ps1[:], thr_sb, 2.0, Alu.is_ge, Alu.mult
                        )  # {0, 2}
                    ps2 = ps2_bufs[gi % 4]
                    dstc = gcp[:, tau * TW : (tau + 1) * TW]
                    nc.tensor.matmul(
                        out=ps2[:, 0:TW], lhsT=w2hi_sb, rhs=s_sb[:],
                        start=True, stop=False,
                    )
                    nc.tensor.matmul(
                        out=ps2[:, 0:TW], lhsT=w2lo_sb, rhs=s_sb[:],
                        start=False, stop=True,
                    )
                    if gi % 3 == 2:
                        nc.vector.tensor_copy(out=dstc, in_=ps2[:, 0:TW])
                    else:
                        nc.scalar.copy(out=dstc, in_=ps2[:, 0:TW])
                # interleave phase-0 horner ahead of the stores so a
                # waiting horner op never blocks the next store in the
                # in-order engine queues
                if t >= 8 and pending:
                    nops = 3 if t < 15 else len(pending)
                    for op in pending[:nops]:
                        op()
                    pending = pending[nops:]
                # store this chunk's gcp to g3
                nc.gpsimd.dma_start(
                    out=g3_st[H, 0, tt], in_=gcp[0:16, :]
                )
                nc.gpsimd.dma_start(
                    out=g3_st[H, 1, tt], in_=gcp[16:32, :]
                )

            for op in pending:
                op()

            # ---- phase 1: remaining loads (tt >= 4) + horner tail ----
            for cd in range(2):
                for k in range(4):
                    nc.sync.dma_start(
                        out=g_all[64:128, cd, k, TW : 2 * TW],
                        in_=g3_dram.ap()[1, cd, k, 4:8],
                    )
            for op in horner_ops(1):
                op()

    nc.compile()
    return nc


def get_program():
    if "prog" not in _PROG_CACHE:
        _PROG_CACHE["prog"] = _build_program()
    return _PROG_CACHE["prog"]


def make_in_maps(x: np.ndarray, coefs: np.ndarray):
    cpack = _tables(coefs)
    shards = np.asarray(x, np.float32).reshape(N_CORES, N)
    return [
        {"x": shards[i].copy(), "cpack": cpack}
        for i in range(N_CORES)
    ]


def kernel(x, coefs, knot_vector=None, _trace: bool = False):
    from concourse.bass_utils import run_bass_kernel_spmd

    nc = get_program()
    in_maps = make_in_maps(x, coefs)
    res = run_bass_kernel_spmd(nc, in_maps, list(range(N_CORES)), trace=_trace)
    out = np.concatenate([r["out"] for r in res.results])
    if _trace:
        return out, res
    return out


# revision 38
# speedup vs baseline: 1.0043x; 1.0043x over previous
"""Cubic B-spline evaluation (uniform knots) on 8 Trainium2 NeuronCores.

v7: qf-based two-phase pipeline.  Spline pair index q = floor(x/2) in [1,31];
on pair q the spline is a cubic in v = x - 2q in [0,2):
  out = HC(v) + rr * HD(v),  rr = 1{v >= 1}
with 32-entry tables C, D (host-derived).  Lookups are step sums over
thresholds 1{qf >= i - 0.5} built by a K=5 bf16 matmul (MM1) over 4 point
slots, an indicator pass (ACT sign / DVE is_ge), and a contraction (MM2)
with bf16 hi+lo difference weights.  Table octets move PSUM->SBUF via the
hi+lo merge op itself (ACT copy after PSUM accumulation, or DVE add of two
column blocks), then transpose to pointwise layout through DRAM scratch.

Two f-phases: chunks 0-7 produce g for point-columns [0,512) (all 128
partitions), chunks 8-15 for [512,1024).  Phase-0 g loads issue at chunk 8
and the phase-0 Horner interleaves with chunks 9-15, so only the phase-1
Horner remains as tail.

Layout (per core, N = 131072): pointwise x_pw[p, f] = x[1024 p + f],
p = 32 s + q.  Group (t, tau): phase H = t//8, tt = t%8, q = 4 tt + tau,
covers points (p = 32 s + q, f = 512 H + c), c in [0,512).
"""

import sys

sys.path.insert(0, "/opt/trn_rl_repo")

import numpy as np

N_TOTAL = 1_048_576
N_CORES = 8
N = N_TOTAL // N_CORES  # 131072 points per core
P = 128
COLS = N // P  # 1024
TW = 512
NCHUNK = 16


def _eng_of_q(q: int) -> str:
    return "act" if (3 * q) % 5 < 3 else "dve"


def _gamma_vec(gamma_k: np.ndarray) -> np.ndarray:
    g = np.zeros((P, 8), np.float32)
    for p in range(P):
        q = 4 * (p // 16) + (p % 4)  # producing group of partition p
        if _eng_of_q(q) == "act":
            g[p] = gamma_k
    return g


def _tables(coefs: np.ndarray):
    import ml_dtypes

    c = np.zeros(67, np.float64)
    c[3:] = np.asarray(coefs, np.float64)
    jj = np.arange(64)
    a0 = (c[jj] + 4 * c[jj + 1] + c[jj + 2]) / 6
    a1 = (c[jj + 2] - c[jj]) / 2
    a2 = (c[jj] - 2 * c[jj + 1] + c[jj + 2]) / 2
    a3 = (c[jj + 3] - c[jj] + 3 * c[jj + 1] - 3 * c[jj + 2]) / 6
    A = np.stack([a0, a1, a2, a3], 1)  # [64, 4] coeffs in u = x - j

    # rebase odd segments to v = u + 1 (v = x - 2q)
    B = A.copy()
    r1 = jj % 2 == 1
    B[r1, 0] = A[r1, 0] - A[r1, 1] + A[r1, 2] - A[r1, 3]
    B[r1, 1] = A[r1, 1] - 2 * A[r1, 2] + 3 * A[r1, 3]
    B[r1, 2] = A[r1, 2] - 3 * A[r1, 3]
    B[r1, 3] = A[r1, 3]
    C = B[0::2]  # [32, 4]
    D = B[1::2] - B[0::2]  # [32, 4]

    # halved step-difference weights (unified sign/{0,2} convention)
    WC = C.copy()
    WC[1:] -= C[:-1]
    WD = D.copy()
    WD[1:] -= D[:-1]
    Wp = np.concatenate([WC, WD], 1) * 0.5  # [32, 8]: col 4 cd + k
    gamma_k = Wp.sum(0).astype(np.float32)  # [8]

    # MM1 lhsT [4, 128]: col 32 s + i -> psum = qf_s; thr applied in the
    # indicator op as a per-partition scalar
    w1 = np.zeros((4, 128), np.float64)
    thr = np.empty(32)
    thr[0] = -1.0
    thr[1:] = np.arange(1, 32) - 0.5
    for s in range(4):
        w1[s, 32 * s : 32 * s + 32] = 1.0
    # MM2 lhsT [128, 32]: row 32 r + i, col m2 = 16 cd + 4 k + r
    w2 = np.zeros((128, 32), np.float64)
    for r in range(4):
        for cd in range(2):
            for k in range(4):
                w2[32 * r : 32 * r + 32, 16 * cd + 4 * k + r] = Wp[:, 4 * cd + k]
    bf = ml_dtypes.bfloat16
    w2hi = w2.astype(bf)
    w2lo = (w2 - w2hi.astype(np.float64)).astype(bf)
    # pack all constants into one [128, 208] bf16 tensor:
    #   cols   0:128  rows 0:5   w1
    #   cols 128:160  w2hi, cols 160:192  w2lo
    #   cols 192:208  gamma (f32 bit-packed as bf16 pairs)
    pack = np.zeros((128, 212), bf)
    pack[0:4, 0:128] = w1.astype(bf)
    pack[:, 128:160] = w2hi
    pack[:, 160:192] = w2lo
    gvec = _gamma_vec(gamma_k)  # [128, 8] f32
    pack[:, 192:208] = gvec.astype(np.float32).view(np.uint16).view(bf)
    thrv = np.tile(thr, 4).astype(np.float32)  # [128] per-partition
    negthr = (-thrv).astype(np.float32)
    pack[:, 208:210] = thrv.reshape(128, 1).view(np.uint16).view(bf)
    pack[:, 210:212] = negthr.reshape(128, 1).view(np.uint16).view(bf)
    return pack


_PROG_CACHE: dict = {}


def _build_program():
    import concourse.bacc as bacc
    import concourse.mybir as mybir
    from concourse.tile import TileContext

    f32 = mybir.dt.float32
    bf16 = mybir.dt.bfloat16
    Alu = mybir.AluOpType

    nc = bacc.Bacc("TRN2", debug=False)

    x_dram = nc.dram_tensor("x", [N], f32, kind="ExternalInput")
    cpack_dram = nc.dram_tensor("cpack", [P, 212], bf16, kind="ExternalInput")
    out_dram = nc.dram_tensor("out", [N], f32, kind="ExternalOutput")
    qf_dram = nc.dram_tensor("qf_scratch", [2, P * TW], bf16, kind="Internal")
    # g3[H, cd, k, tt, r, tau, fpc]
    g3_dram = nc.dram_tensor(
        "g_scratch", [2, 2, 4, 8, 4, 4, TW], f32, kind="Internal"
    )

    x_view = x_dram.ap().rearrange("(p f) -> p f", p=P)
    out_view = out_dram.ap().rearrange("(p f) -> p f", p=P)
    # qf loads per chunk pair: [H, tp, r, ttsub, (tau fpc)=2048]
    qf_ld = qf_dram.ap().rearrange(
        "H (tp ttsub r tf) -> H tp r ttsub tf", tp=4, ttsub=2, r=4
    )
    # g3 store view: [H, cd, tt, k, r, (tau fpc)]
    g3_st = g3_dram.ap().rearrange(
        "H cd k tt r tau fpc -> H cd tt k r (tau fpc)"
    )

    with TileContext(nc) as tc:
        with (
            tc.tile_pool(name="const", bufs=1) as cpool,
            tc.tile_pool(name="pw", bufs=1) as pw,
            tc.tile_pool(name="tmp", bufs=1) as tmp,
            tc.tile_pool(name="hrn", bufs=1) as hp,
            tc.tile_pool(name="sind", bufs=1) as spool,
            tc.tile_pool(name="gcp", bufs=1) as gcpool,
            tc.tile_pool(name="psum1", bufs=1, space="PSUM") as pp1,
            tc.tile_pool(name="psum2", bufs=1, space="PSUM") as pp2,
        ):
            # ---- constants: one packed DMA; ones rows via pool memset ----
            cpk = cpool.tile([P, 212], bf16, tag="cpk")
            nc.sync.dma_start(out=cpk[:], in_=cpack_dram.ap())
            w1_sb = cpk[0:4, 0:128]
            w2hi_sb = cpk[:, 128:160]
            w2lo_sb = cpk[:, 160:192]
            gam_sb = cpk[:, 192:208].bitcast(f32)
            thr_sb = cpk[:, 208:210].bitcast(f32)
            negthr_sb = cpk[:, 210:212].bitcast(f32)
            j_bufs = []
            for bi in range(3):
                jb = cpool.tile(
                    [4, 4096], bf16, tag=f"jbuf{bi}", name=f"jbuf{bi}"
                )
                j_bufs.append(jb)

            ps1_bufs = [
                pp1.tile([P, TW], f32, tag=f"s1_{i}", name=f"ps1f{i}")
                for i in range(4)
            ]
            ps2_bufs = [
                pp2.tile([32, TW], f32, tag=f"s2_{i}", name=f"ps2f{i}")
                for i in range(4)
            ]
            s_bufs = [
                spool.tile([P, TW], bf16, tag=f"sb_{i}", name=f"sbf{i}")
                for i in range(8)
            ]
            gcp_full = [
                gcpool.tile([32, 4 * TW], f32, tag=f"gc_{i}", name=f"gcpf{i}")
                for i in range(3)
            ]

            # dummies: absorb constant-load DMA sems into the PE vector clock
            pdum = ps1_bufs[0]
            nc.tensor.matmul(
                out=pdum[:, 0:8], lhsT=w1_sb[:], rhs=w1_sb[:, 0:8],
                start=True, stop=True,
            )
            nc.tensor.matmul(
                out=pdum[0:32, 0:8], lhsT=w2hi_sb[:], rhs=w2hi_sb[:, 0:8],
                start=True, stop=True,
            )
            nc.tensor.matmul(
                out=pdum[0:32, 0:8], lhsT=w2lo_sb[:], rhs=w2lo_sb[:, 0:8],
                start=True, stop=True,
            )

            # ---- pointwise prep, split by phase half so chunk 0 can
            # start as soon as the H=0 qf scratch is written ----
            x_pw = pw.tile([P, COLS], f32, tag="x")
            t1_pw = tmp.tile([P, COLS], f32, tag="ta", name="prep_t1")
            qf_pw = pw.tile([P, COLS], bf16, tag="qf")
            v_pw = pw.tile([P, COLS], f32, tag="v")
            rr_pw = pw.tile([P, COLS], f32, tag="rr")
            v2_pw = pw.tile([P, COLS], f32, tag="v2")
            for hh in range(2):
                sl = slice(TW * hh, TW * hh + TW)
                nc.sync.dma_start(out=x_pw[:, sl], in_=x_view[:, sl])
                nc.vector.tensor_scalar(
                    t1_pw[:, sl], x_pw[:, sl], 0.5, 8388607.5,
                    Alu.mult, Alu.add
                )
                nc.vector.tensor_scalar(
                    qf_pw[:, sl], t1_pw[:, sl], -8388608.0, None, Alu.add
                )
                nc.sync.dma_start(
                    out=qf_dram.ap()[hh].rearrange("(p c) -> p c", p=P),
                    in_=qf_pw[:, sl],
                )
                nc.vector.scalar_tensor_tensor(
                    v_pw[:, sl], qf_pw[:, sl], -2.0, x_pw[:, sl],
                    Alu.mult, Alu.add
                )
                nc.vector.tensor_scalar(
                    rr_pw[:, sl], v_pw[:, sl], 1.0, None, Alu.is_ge
                )
                nc.scalar.square(v2_pw[:, sl], v_pw[:, sl])

            g_all = pw.tile([P, 2, 4, COLS], f32, tag="gall")
            res = pw.tile([P, COLS], f32, tag="res")

            # ---- phase Horner op lists (emitted interleaved) ----
            def horner_ops(H):
                c0 = TW * H
                sl = slice(c0, c0 + TW)
                v_ = v_pw[:, sl]
                v2_ = v2_pw[:, sl]
                rr_ = rr_pw[:, sl]
                ops = []
                hres = []
                for cd in range(2):
                    gk = [g_all[:, cd, k, sl] for k in range(4)]
                    g2c = hp.tile([P, TW], f32, tag=f"h{cd}a", name=f"g2c{cd}_{H}")
                    g3c = hp.tile([P, TW], f32, tag=f"h{cd}b", name=f"g3c{cd}_{H}")
                    v1t = hp.tile([P, TW], f32, tag=f"h{cd}c", name=f"v1t{cd}_{H}")
                    v2t = hp.tile([P, TW], f32, tag=f"h{cd}d", name=f"v2t{cd}_{H}")
                    pacc = hp.tile([P, TW], f32, tag=f"h{cd}a", name=f"pacc{cd}_{H}")
                    qacc = hp.tile([P, TW], f32, tag=f"h{cd}b", name=f"qacc{cd}_{H}")
                    v3t = hp.tile([P, TW], f32, tag=f"h{cd}c", name=f"v3t{cd}_{H}")
                    hr = hp.tile([P, TW], f32, tag=f"h{cd}d", name=f"hr{cd}_{H}")
                    e = nc.vector if cd == 0 else nc.gpsimd
                    ops.append(lambda gk=gk, g2c=g2c, cd=cd: nc.scalar.add(
                        g2c[:], gk[2], gam_sb[:, 4 * cd + 2 : 4 * cd + 3]))
                    ops.append(lambda gk=gk, g3c=g3c, cd=cd: nc.scalar.add(
                        g3c[:], gk[3], gam_sb[:, 4 * cd + 3 : 4 * cd + 4]))
                    ops.append(lambda e=e, v1t=v1t, g2c=g2c, v2_=v2_:
                               e.tensor_tensor(out=v1t[:], in0=g2c[:],
                                               in1=v2_, op=Alu.mult))
                    ops.append(lambda e=e, v2t=v2t, g3c=g3c, v2_=v2_:
                               e.tensor_tensor(out=v2t[:], in0=g3c[:],
                                               in1=v2_, op=Alu.mult))
                    ops.append(lambda pacc=pacc, v1t=v1t, gk=gk, cd=cd:
                               nc.vector.scalar_tensor_tensor(
                                   pacc[:], v1t[:],
                                   gam_sb[:, 4 * cd : 4 * cd + 1], gk[0],
                                   Alu.add, Alu.add))
                    ops.append(lambda qacc=qacc, v2t=v2t, gk=gk, cd=cd:
                               nc.vector.scalar_tensor_tensor(
                                   qacc[:], v2t[:],
                                   gam_sb[:, 4 * cd + 1 : 4 * cd + 2], gk[1],
                                   Alu.add, Alu.add))
                    ops.append(lambda e=e, v3t=v3t, qacc=qacc, v_=v_:
                               e.tensor_tensor(out=v3t[:], in0=qacc[:],
                                               in1=v_, op=Alu.mult))
                    ops.append(lambda e=e, hr=hr, pacc=pacc, v3t=v3t:
                               e.tensor_tensor(out=hr[:], in0=pacc[:],
                                               in1=v3t[:], op=Alu.add))
                    hres.append(hr)
                rd = hp.tile([P, TW], f32, tag="h0a", name=f"rd_{H}")
                ops.append(lambda rd=rd, h1=hres[1], rr_=rr_:
                           nc.vector.tensor_tensor(out=rd[:], in0=h1[:],
                                                   in1=rr_, op=Alu.mult))
                ops.append(lambda rd=rd, h0=hres[0], sl=sl:
                           nc.vector.tensor_tensor(out=res[:, sl], in0=h0[:],
                                                   in1=rd[:], op=Alu.add))
                ops.append(lambda sl=sl: nc.sync.dma_start(
                    out=out_view[:, sl], in_=res[:, sl]))
                return ops

            pending = []

            # ---- chunk loop ----
            for t in range(NCHUNK):
                H, tt = t // 8, t % 8
                if 4 <= t < 8 or 12 <= t < 16:
                    # early loads: first tt-half of the current phase,
                    # spread 2 per chunk to keep HWDGE smooth
                    Hc = t // 8
                    for k2 in range(2):
                        ldi = 2 * (t % 4) + k2
                        cd, k = ldi // 4, ldi % 4
                        nc.sync.dma_start(
                            out=g_all[0:64, cd, k,
                                      TW * Hc : TW * Hc + TW],
                            in_=g3_dram.ap()[Hc, cd, k, 0:4],
                        )
                if 8 <= t < 12:
                    # phase-0 remaining loads (tt >= 4), 2 per chunk
                    for k2 in range(2):
                        ldi = 2 * (t % 4) + k2
                        cd, k = ldi // 4, ldi % 4
                        nc.sync.dma_start(
                            out=g_all[64:128, cd, k, 0:TW],
                            in_=g3_dram.ap()[0, cd, k, 4:8],
                        )
                if t == 8:
                    pending = horner_ops(0)
                j_pk = j_bufs[(t // 2) % 3]
                if t == 0:
                    # pair 0 load (pair i+1 is prefetched a pair ahead below)
                    nc.sync.dma_start(
                        out=j_pk[0:4, :], in_=qf_ld[0, 0]
                    )
                if t % 2 == 0 and t < NCHUNK - 2:
                    pn = t // 2 + 1
                    nc.sync.dma_start(
                        out=j_bufs[pn % 3][0:4, :],
                        in_=qf_ld[pn // 4, pn % 4],
                    )
                gcp = gcp_full[t % 3]
                for tau in range(4):
                    q = 4 * tt + tau
                    gi = 4 * t + tau
                    eng = _eng_of_q(q)
                    ps1 = ps1_bufs[gi % 4]
                    nc.tensor.matmul(
                        out=ps1[:],
                        lhsT=w1_sb[:],
                        rhs=j_pk[
                            :, (t % 2) * 2048 + tau * TW :
                            (t % 2) * 2048 + (tau + 1) * TW
                        ],
                        start=True,
                        stop=True,
                    )
                    s_sb = s_bufs[gi % 8]
                    if eng == "act":
                        nc.scalar.sign(s_sb[:], ps1[:], bias=negthr_sb)  # {-1,+1}
                    else:
                        nc.vector.tensor_scalar(
                            s_sb[:], ps1[:], thr_sb, 2.0, Alu.is_ge, Alu.mult
                        )  # {0, 2}
                    ps2 = ps2_bufs[gi % 4]
                    dstc = gcp[:, tau * TW : (tau + 1) * TW]
                    nc.tensor.matmul(
                        out=ps2[:, 0:TW], lhsT=w2hi_sb, rhs=s_sb[:],
                        start=True, stop=False,
                    )
                    nc.tensor.matmul(
                        out=ps2[:, 0:TW], lhsT=w2lo_sb, rhs=s_sb[:],
                        start=False, stop=True,
                    )
                    if gi % 3 == 2:
                        nc.vector.tensor_copy(out=dstc, in_=ps2[:, 0:TW])
                    else:
                        nc.scalar.copy(out=dstc, in_=ps2[:, 0:TW])
                # store this chunk's gcp to g3 (cd 0 via pool, cd 1 via SP)
                nc.gpsimd.dma_start(
                    out=g3_st[H, 0, tt], in_=gcp[0:16, :]
                )
                nc.gpsimd.dma_start(
                    out=g3_st[H, 1, tt], in_=gcp[16:32, :]
                )
                # interleave phase-0 horner into chunks 9..15
                if t >= 9 and pending:
                    nops = 3 if t < 15 else len(pending)
                    for op in pending[:nops]:
                        op()
                    pending = pending[nops:]

            for op in pending:
                op()

            # ---- phase 1: remaining loads (tt >= 4) + horner tail ----
            for cd in range(2):
                for k in range(4):
                    nc.sync.dma_start(
                        out=g_all[64:128, cd, k, TW : 2 * TW],
                        in_=g3_dram.ap()[1, cd, k, 4:8],
                    )
            for op in horner_ops(1):
                op()

    nc.compile()
    return nc


def get_program():
    if "prog" not in _PROG_CACHE:
        _PROG_CACHE["prog"] = _build_program()
    return _PROG_CACHE["prog"]


def make_in_maps(x: np.ndarray, coefs: np.ndarray):
    cpack = _tables(coefs)
    shards = np.asarray(x, np.float32).reshape(N_CORES, N)
    return [
        {"x": shards[i].copy(), "cpack": cpack}
        for i in range(N_CORES)
    ]


def kernel(x, coefs, knot_vector=None, _trace: bool = False):
    from concourse.bass_utils import run_bass_kernel_spmd

    nc = get_program()
    in_maps = make_in_maps(x, coefs)
    res = run_bass_kernel_spmd(nc, in_maps, list(range(N_CORES)), trace=_trace)
    out = np.concatenate([r["out"] for r in res.results])
    if _trace:
        return out, res
    return out


# revision 39
# speedup vs baseline: 1.0322x; 1.0278x over previous
"""Cubic B-spline evaluation (uniform knots) on 8 Trainium2 NeuronCores.

v7: qf-based two-phase pipeline.  Spline pair index q = floor(x/2) in [1,31];
on pair q the spline is a cubic in v = x - 2q in [0,2):
  out = HC(v) + rr * HD(v),  rr = 1{v >= 1}
with 32-entry tables C, D (host-derived).  Lookups are step sums over
thresholds 1{qf >= i - 0.5} built by a K=5 bf16 matmul (MM1) over 4 point
slots, an indicator pass (ACT sign / DVE is_ge), and a contraction (MM2)
with bf16 hi+lo difference weights.  Table octets move PSUM->SBUF via the
hi+lo merge op itself (ACT copy after PSUM accumulation, or DVE add of two
column blocks), then transpose to pointwise layout through DRAM scratch.

Two f-phases: chunks 0-7 produce g for point-columns [0,512) (all 128
partitions), chunks 8-15 for [512,1024).  Phase-0 g loads issue at chunk 8
and the phase-0 Horner interleaves with chunks 9-15, so only the phase-1
Horner remains as tail.

Layout (per core, N = 131072): pointwise x_pw[p, f] = x[1024 p + f],
p = 32 s + q.  Group (t, tau): phase H = t//8, tt = t%8, q = 4 tt + tau,
covers points (p = 32 s + q, f = 512 H + c), c in [0,512).
"""

import sys

sys.path.insert(0, "/opt/trn_rl_repo")

import numpy as np

N_TOTAL = 1_048_576
N_CORES = 8
N = N_TOTAL // N_CORES  # 131072 points per core
P = 128
COLS = N // P  # 1024
TW = 512
NCHUNK = 16


def _eng_of_q(q: int) -> str:
    return "act" if q % 2 == 0 else "dve"


def _gamma_vec(gamma_k: np.ndarray) -> np.ndarray:
    g = np.zeros((P, 8), np.float32)
    for p in range(P):
        q = 4 * (p // 16) + (p % 4)  # producing group of partition p
        if _eng_of_q(q) == "act":
            g[p] = gamma_k
    return g


def _tables(coefs: np.ndarray):
    import ml_dtypes

    c = np.zeros(67, np.float64)
    c[3:] = np.asarray(coefs, np.float64)
    jj = np.arange(64)
    a0 = (c[jj] + 4 * c[jj + 1] + c[jj + 2]) / 6
    a1 = (c[jj + 2] - c[jj]) / 2
    a2 = (c[jj] - 2 * c[jj + 1] + c[jj + 2]) / 2
    a3 = (c[jj + 3] - c[jj] + 3 * c[jj + 1] - 3 * c[jj + 2]) / 6
    A = np.stack([a0, a1, a2, a3], 1)  # [64, 4] coeffs in u = x - j

    # rebase odd segments to v = u + 1 (v = x - 2q)
    B = A.copy()
    r1 = jj % 2 == 1
    B[r1, 0] = A[r1, 0] - A[r1, 1] + A[r1, 2] - A[r1, 3]
    B[r1, 1] = A[r1, 1] - 2 * A[r1, 2] + 3 * A[r1, 3]
    B[r1, 2] = A[r1, 2] - 3 * A[r1, 3]
    B[r1, 3] = A[r1, 3]
    C = B[0::2]  # [32, 4]
    D = B[1::2] - B[0::2]  # [32, 4]

    # halved step-difference weights (unified sign/{0,2} convention)
    WC = C.copy()
    WC[1:] -= C[:-1]
    WD = D.copy()
    WD[1:] -= D[:-1]
    Wp = np.concatenate([WC, WD], 1) * 0.5  # [32, 8]: col 4 cd + k
    gamma_k = Wp.sum(0).astype(np.float32)  # [8]

    # MM1 lhsT [4, 128]: col 32 s + i -> psum = qf_s; thr applied in the
    # indicator op as a per-partition scalar
    w1 = np.zeros((4, 128), np.float64)
    thr = np.empty(32)
    thr[0] = -1.0
    thr[1:] = np.arange(1, 32) - 0.5
    for s in range(4):
        w1[s, 32 * s : 32 * s + 32] = 1.0
    # MM2 lhsT [128, 32]: row 32 r + i, col m2 = 16 cd + 4 k + r
    w2 = np.zeros((128, 32), np.float64)
    for r in range(4):
        for cd in range(2):
            for k in range(4):
                w2[32 * r : 32 * r + 32, 16 * cd + 4 * k + r] = Wp[:, 4 * cd + k]
    bf = ml_dtypes.bfloat16
    w2hi = w2.astype(bf)
    w2lo = (w2 - w2hi.astype(np.float64)).astype(bf)
    # pack all constants into one [128, 208] bf16 tensor:
    #   cols   0:128  rows 0:5   w1
    #   cols 128:160  w2hi, cols 160:192  w2lo
    #   cols 192:208  gamma (f32 bit-packed as bf16 pairs)
    pack = np.zeros((128, 212), bf)
    pack[0:4, 0:128] = w1.astype(bf)
    pack[:, 128:160] = w2hi
    pack[:, 160:192] = w2lo
    gvec = _gamma_vec(gamma_k)  # [128, 8] f32
    pack[:, 192:208] = gvec.astype(np.float32).view(np.uint16).view(bf)
    thrv = np.tile(thr, 4).astype(np.float32)  # [128] per-partition
    negthr = (-thrv).astype(np.float32)
    pack[:, 208:210] = thrv.reshape(128, 1).view(np.uint16).view(bf)
    pack[:, 210:212] = negthr.reshape(128, 1).view(np.uint16).view(bf)
    return pack


_PROG_CACHE: dict = {}


def _build_program():
    import concourse.bacc as bacc
    import concourse.mybir as mybir
    from concourse.tile import TileContext

    f32 = mybir.dt.float32
    bf16 = mybir.dt.bfloat16
    Alu = mybir.AluOpType

    nc = bacc.Bacc("TRN2", debug=False)

    x_dram = nc.dram_tensor("x", [N], f32, kind="ExternalInput")
    cpack_dram = nc.dram_tensor("cpack", [P, 212], bf16, kind="ExternalInput")
    out_dram = nc.dram_tensor("out", [N], f32, kind="ExternalOutput")
    qf_dram = nc.dram_tensor("qf_scratch", [2, P * TW], bf16, kind="Internal")
    # g3[H, cd, k, tt, r, tau, fpc]
    g3_dram = nc.dram_tensor(
        "g_scratch", [2, 2, 4, 8, 4, 4, TW], f32, kind="Internal"
    )

    x_view = x_dram.ap().rearrange("(p f) -> p f", p=P)
    out_view = out_dram.ap().rearrange("(p f) -> p f", p=P)
    # qf loads per chunk pair: [H, tp, r, ttsub, (tau fpc)=2048]
    qf_ld = qf_dram.ap().rearrange(
        "H (tp ttsub r tf) -> H tp r ttsub tf", tp=4, ttsub=2, r=4
    )
    # g3 store view: [H, cd, tt, k, r, (tau fpc)]
    g3_st = g3_dram.ap().rearrange(
        "H cd k tt r tau fpc -> H cd tt k r (tau fpc)"
    )

    with TileContext(nc) as tc:
        with (
            tc.tile_pool(name="const", bufs=1) as cpool,
            tc.tile_pool(name="pw", bufs=1) as pw,
            tc.tile_pool(name="tmp", bufs=1) as tmp,
            tc.tile_pool(name="hrn", bufs=1) as hp,
            tc.tile_pool(name="sind", bufs=1) as spool,
            tc.tile_pool(name="gcp", bufs=1) as gcpool,
            tc.tile_pool(name="psum1", bufs=1, space="PSUM") as pp1,
            tc.tile_pool(name="psum2", bufs=1, space="PSUM") as pp2,
        ):
            # ---- constants: one packed DMA; ones rows via pool memset ----
            cpk = cpool.tile([P, 212], bf16, tag="cpk")
            nc.sync.dma_start(out=cpk[:], in_=cpack_dram.ap())
            w1_sb = cpk[0:4, 0:128]
            w2hi_sb = cpk[:, 128:160]
            w2lo_sb = cpk[:, 160:192]
            gam_sb = cpk[:, 192:208].bitcast(f32)
            thr_sb = cpk[:, 208:210].bitcast(f32)
            negthr_sb = cpk[:, 210:212].bitcast(f32)
            j_bufs = []
            for bi in range(3):
                jb = cpool.tile(
                    [4, 4096], bf16, tag=f"jbuf{bi}", name=f"jbuf{bi}"
                )
                j_bufs.append(jb)

            ps1_bufs = [
                pp1.tile([P, TW], f32, tag=f"s1_{i}", name=f"ps1f{i}")
                for i in range(4)
            ]
            ps2_bufs = [
                pp2.tile([32, TW], f32, tag=f"s2_{i}", name=f"ps2f{i}")
                for i in range(4)
            ]
            s_bufs = [
                spool.tile([P, TW], bf16, tag=f"sb_{i}", name=f"sbf{i}")
                for i in range(8)
            ]
            gcp_full = [
                gcpool.tile([32, 4 * TW], f32, tag=f"gc_{i}", name=f"gcpf{i}")
                for i in range(3)
            ]

            # dummies: absorb constant-load DMA sems into the PE vector clock
            pdum = ps1_bufs[0]
            nc.tensor.matmul(
                out=pdum[:, 0:8], lhsT=w1_sb[:], rhs=w1_sb[:, 0:8],
                start=True, stop=True,
            )
            nc.tensor.matmul(
                out=pdum[0:32, 0:8], lhsT=w2hi_sb[:], rhs=w2hi_sb[:, 0:8],
                start=True, stop=True,
            )
            nc.tensor.matmul(
                out=pdum[0:32, 0:8], lhsT=w2lo_sb[:], rhs=w2lo_sb[:, 0:8],
                start=True, stop=True,
            )

            # ---- pointwise prep, split by phase half so chunk 0 can
            # start as soon as the H=0 qf scratch is written ----
            x_pw = pw.tile([P, COLS], f32, tag="x")
            t1_pw = tmp.tile([P, COLS], f32, tag="ta", name="prep_t1")
            qf_pw = pw.tile([P, COLS], bf16, tag="qf")
            v_pw = pw.tile([P, COLS], f32, tag="v")
            rr_pw = pw.tile([P, COLS], f32, tag="rr")
            v2_pw = pw.tile([P, COLS], f32, tag="v2")
            for hh in range(2):
                sl = slice(TW * hh, TW * hh + TW)
                nc.sync.dma_start(out=x_pw[:, sl], in_=x_view[:, sl])
                nc.vector.tensor_scalar(
                    t1_pw[:, sl], x_pw[:, sl], 0.5, 8388607.5,
                    Alu.mult, Alu.add
                )
                nc.vector.tensor_scalar(
                    qf_pw[:, sl], t1_pw[:, sl], -8388608.0, None, Alu.add
                )
                nc.sync.dma_start(
                    out=qf_dram.ap()[hh].rearrange("(p c) -> p c", p=P),
                    in_=qf_pw[:, sl],
                )
                nc.vector.scalar_tensor_tensor(
                    v_pw[:, sl], qf_pw[:, sl], -2.0, x_pw[:, sl],
                    Alu.mult, Alu.add
                )
                nc.vector.tensor_scalar(
                    rr_pw[:, sl], v_pw[:, sl], 1.0, None, Alu.is_ge
                )
                nc.scalar.square(v2_pw[:, sl], v_pw[:, sl])

            g_all = pw.tile([P, 2, 4, COLS], f32, tag="gall")
            res = pw.tile([P, COLS], f32, tag="res")

            # ---- phase Horner op lists (emitted interleaved) ----
            def horner_ops(H):
                c0 = TW * H
                sl = slice(c0, c0 + TW)
                v_ = v_pw[:, sl]
                v2_ = v2_pw[:, sl]
                rr_ = rr_pw[:, sl]
                ops = []
                hres = []
                for cd in range(2):
                    gk = [g_all[:, cd, k, sl] for k in range(4)]
                    g2c = hp.tile([P, TW], f32, tag=f"h{cd}a", name=f"g2c{cd}_{H}")
                    g3c = hp.tile([P, TW], f32, tag=f"h{cd}b", name=f"g3c{cd}_{H}")
                    v1t = hp.tile([P, TW], f32, tag=f"h{cd}c", name=f"v1t{cd}_{H}")
                    v2t = hp.tile([P, TW], f32, tag=f"h{cd}d", name=f"v2t{cd}_{H}")
                    pacc = hp.tile([P, TW], f32, tag=f"h{cd}a", name=f"pacc{cd}_{H}")
                    qacc = hp.tile([P, TW], f32, tag=f"h{cd}b", name=f"qacc{cd}_{H}")
                    v3t = hp.tile([P, TW], f32, tag=f"h{cd}c", name=f"v3t{cd}_{H}")
                    hr = hp.tile([P, TW], f32, tag=f"h{cd}d", name=f"hr{cd}_{H}")
                    e = nc.vector if cd == 0 else nc.gpsimd
                    ops.append(lambda gk=gk, g2c=g2c, cd=cd: nc.scalar.add(
                        g2c[:], gk[2], gam_sb[:, 4 * cd + 2 : 4 * cd + 3]))
                    ops.append(lambda gk=gk, g3c=g3c, cd=cd: nc.scalar.add(
                        g3c[:], gk[3], gam_sb[:, 4 * cd + 3 : 4 * cd + 4]))
                    ops.append(lambda e=e, v1t=v1t, g2c=g2c, v2_=v2_:
                               e.tensor_tensor(out=v1t[:], in0=g2c[:],
                                               in1=v2_, op=Alu.mult))
                    ops.append(lambda e=e, v2t=v2t, g3c=g3c, v2_=v2_:
                               e.tensor_tensor(out=v2t[:], in0=g3c[:],
                                               in1=v2_, op=Alu.mult))
                    ops.append(lambda pacc=pacc, v1t=v1t, gk=gk, cd=cd:
                               nc.vector.scalar_tensor_tensor(
                                   pacc[:], v1t[:],
                                   gam_sb[:, 4 * cd : 4 * cd + 1], gk[0],
                                   Alu.add, Alu.add))
                    ops.append(lambda qacc=qacc, v2t=v2t, gk=gk, cd=cd:
                               nc.vector.scalar_tensor_tensor(
                                   qacc[:], v2t[:],
                                   gam_sb[:, 4 * cd + 1 : 4 * cd + 2], gk[1],
                                   Alu.add, Alu.add))
                    ops.append(lambda e=e, v3t=v3t, qacc=qacc, v_=v_:
                               e.tensor_tensor(out=v3t[:], in0=qacc[:],
                                               in1=v_, op=Alu.mult))
                    ops.append(lambda e=e, hr=hr, pacc=pacc, v3t=v3t:
                               e.tensor_tensor(out=hr[:], in0=pacc[:],
                                               in1=v3t[:], op=Alu.add))
                    hres.append(hr)
                rd = hp.tile([P, TW], f32, tag="h0a", name=f"rd_{H}")
                ops.append(lambda rd=rd, h1=hres[1], rr_=rr_:
                           nc.vector.tensor_tensor(out=rd[:], in0=h1[:],
                                                   in1=rr_, op=Alu.mult))
                ops.append(lambda rd=rd, h0=hres[0], sl=sl:
                           nc.vector.tensor_tensor(out=res[:, sl], in0=h0[:],
                                                   in1=rd[:], op=Alu.add))
                ops.append(lambda sl=sl: nc.sync.dma_start(
                    out=out_view[:, sl], in_=res[:, sl]))
                return ops

            pending = []

            # ---- chunk loop ----
            for t in range(NCHUNK):
                H, tt = t // 8, t % 8
                if 4 <= t < 8 or 12 <= t < 16:
                    # early loads: first tt-half of the current phase,
                    # spread 2 per chunk to keep HWDGE smooth
                    Hc = t // 8
                    for k2 in range(2):
                        ldi = 2 * (t % 4) + k2
                        cd, k = ldi // 4, ldi % 4
                        nc.sync.dma_start(
                            out=g_all[0:64, cd, k,
                                      TW * Hc : TW * Hc + TW],
                            in_=g3_dram.ap()[Hc, cd, k, 0:4],
                        )
                if 8 <= t < 12:
                    # phase-0 remaining loads (tt >= 4), 2 per chunk
                    for k2 in range(2):
                        ldi = 2 * (t % 4) + k2
                        cd, k = ldi // 4, ldi % 4
                        nc.sync.dma_start(
                            out=g_all[64:128, cd, k, 0:TW],
                            in_=g3_dram.ap()[0, cd, k, 4:8],
                        )
                if t == 8:
                    pending = horner_ops(0)
                j_pk = j_bufs[(t // 2) % 3]
                if t == 0:
                    # pair 0 load (pair i+1 is prefetched a pair ahead below)
                    nc.sync.dma_start(
                        out=j_pk[0:4, :], in_=qf_ld[0, 0]
                    )
                if t % 2 == 0 and t < NCHUNK - 2:
                    pn = t // 2 + 1
                    nc.sync.dma_start(
                        out=j_bufs[pn % 3][0:4, :],
                        in_=qf_ld[pn // 4, pn % 4],
                    )
                gcp = gcp_full[t % 3]
                for tau in range(4):
                    q = 4 * tt + tau
                    gi = 4 * t + tau
                    eng = _eng_of_q(q)
                    ps1 = ps1_bufs[gi % 4]
                    nc.tensor.matmul(
                        out=ps1[:],
                        lhsT=w1_sb[:],
                        rhs=j_pk[
                            :, (t % 2) * 2048 + tau * TW :
                            (t % 2) * 2048 + (tau + 1) * TW
                        ],
                        start=True,
                        stop=True,
                    )
                    s_sb = s_bufs[gi % 8]
                    if eng == "act":
                        nc.scalar.sign(s_sb[:], ps1[:], bias=negthr_sb)  # {-1,+1}
                    else:
                        nc.vector.tensor_scalar(
                            s_sb[:], ps1[:], thr_sb, 2.0, Alu.is_ge, Alu.mult
                        )  # {0, 2}
                    ps2 = ps2_bufs[gi % 4]
                    dstc = gcp[:, tau * TW : (tau + 1) * TW]
                    nc.tensor.matmul(
                        out=ps2[:, 0:TW], lhsT=w2hi_sb, rhs=s_sb[:],
                        start=True, stop=False,
                    )
                    nc.tensor.matmul(
                        out=ps2[:, 0:TW], lhsT=w2lo_sb, rhs=s_sb[:],
                        start=False, stop=True,
                    )
                    if gi % 3 == 2:
                        nc.vector.tensor_copy(out=dstc, in_=ps2[:, 0:TW])
                    else:
                        nc.scalar.copy(out=dstc, in_=ps2[:, 0:TW])
                # store this chunk's gcp to g3 (cd 0 via pool, cd 1 via SP)
                nc.gpsimd.dma_start(
                    out=g3_st[H, 0, tt], in_=gcp[0:16, :]
                )
                nc.gpsimd.dma_start(
                    out=g3_st[H, 1, tt], in_=gcp[16:32, :]
                )
                # interleave phase-0 horner into chunks 9..15
                if t >= 9 and pending:
                    nops = 3 if t < 15 else len(pending)
                    for op in pending[:nops]:
                        op()
                    pending = pending[nops:]

            for op in pending:
                op()

            # ---- phase 1: remaining loads (tt >= 4) + horner tail ----
            for cd in range(2):
                for k in range(4):
                    nc.sync.dma_start(
                        out=g_all[64:128, cd, k, TW : 2 * TW],
                        in_=g3_dram.ap()[1, cd, k, 4:8],
                    )
            for op in horner_ops(1):
                op()

    nc.compile()
    return nc


def get_program():
    if "prog" not in _PROG_CACHE:
        _PROG_CACHE["prog"] = _build_program()
    return _PROG_CACHE["prog"]


def make_in_maps(x: np.ndarray, coefs: np.ndarray):
    cpack = _tables(coefs)
    shards = np.asarray(x, np.float32).reshape(N_CORES, N)
    return [
        {"x": shards[i].copy(), "cpack": cpack}
        for i in range(N_CORES)
    ]


def kernel(x, coefs, knot_vector=None, _trace: bool = False):
    from concourse.bass_utils import run_bass_kernel_spmd

    nc = get_program()
    in_maps = make_in_maps(x, coefs)
    res = run_bass_kernel_spmd(nc, in_maps, list(range(N_CORES)), trace=_trace)
    out = np.concatenate([r["out"] for r in res.results])
    if _trace:
        return out, res
    return out
